# revision 1
# baseline (speedup 1.0000x reference)
"""Trainium2 Bass kernel for nn_AttentionHead (softmax over query axis).

Sharding: 8 cores = 4 batches x 2 halves. Core c handles batch c//2 and
row-parity h=c%2: local 128-row chunk lc <-> global chunk g=2*lc+h.
Per core:
  - cast x rows + weights to bf16 during DMA
  - PE-transpose x tiles -> xT [E-chunk, 1024 t]
  - projections qT/kT [128 D, 1024 t] and vT -> v natural [t, D]
  - AllGather kT, v across the pair (replica groups of 2)
  - scores sT[s, t] = kT_blk.T @ qT, exp (scale 1/sqrt(128)) with per-key
    column sums (softmax normalizer is over the QUERY axis), causal mask
    via host-supplied mask tiles (h=0: [tri, zeros], h=1: [ones, tri])
  - AllReduce the [128,16] normalizer partials across the pair
  - z[t, :] = sum_s E[s,t] * (v[s,:]/Z[s])
Host assembles the 8 core outputs back into [4, 2048, 128].
"""
import sys

for _p in ("/opt/trn_rl_repo",):
    if _p not in sys.path:
        sys.path.append(_p)

import numpy as np
import ml_dtypes

import concourse.bass as bass
import concourse.mybir as mybir
import concourse.tile as tile
from concourse import bacc
from concourse.bass import ds, ts
from concourse.bass_utils import run_bass_kernel_spmd
from concourse.masks import make_identity

BF16 = mybir.dt.bfloat16
F32 = mybir.dt.float32
AF = mybir.ActivationFunctionType
ALU = mybir.AluOpType
AX = mybir.AxisListType

B, T, E, D = 4, 2048, 2048, 128
NLC = 8          # local 128-row chunks per core
NE = 16          # E chunks of 128
NSB = 16         # key blocks of 128
SCALE = 1.0 / np.sqrt(D)
N_CORES = 8
REPLICA_GROUPS = [[0, 1], [2, 3], [4, 5], [6, 7]]


def gpos(g: int) -> int:
    """Global 128-chunk index -> position in the pair-gathered buffer."""
    return (g % 2) * 8 + g // 2


def build_nc():
    nc = bacc.Bacc("TRN2", target_bir_lowering=False, debug=False,
                   num_devices=N_CORES)
    x = nc.dram_tensor("x", [NLC * 128, E], F32, kind="ExternalInput")
    wq = nc.dram_tensor("wq", [E, D], F32, kind="ExternalInput")
    wk = nc.dram_tensor("wk", [E, D], F32, kind="ExternalInput")
    wv = nc.dram_tensor("wv", [E, D], F32, kind="ExternalInput")
    masks = nc.dram_tensor("masks", [128, 2, 128], F32, kind="ExternalInput")
    out = nc.dram_tensor("out", [NLC * 128, D], F32, kind="ExternalOutput")

    with tile.TileContext(nc) as tc:
        _body(nc, tc, x, wq, wk, wv, masks, out)
    nc.compile()
    return nc


def _body(nc, tc, x, wq, wk, wv, masks, out):
    with (
        tc.tile_pool(name="const", bufs=1) as const_pool,
        tc.tile_pool(name="dram", bufs=1, space="DRAM") as dram_pool,
        tc.tile_pool(name="xnat", bufs=NLC) as xnat_pool,
        tc.tile_pool(name="xt", bufs=NE) as xt_pool,
        tc.tile_pool(name="proj", bufs=1) as proj_pool,
        tc.tile_pool(name="escore", bufs=1) as e_pool,
        tc.tile_pool(name="zout", bufs=2) as zout_pool,
    ):
        # ---- constants ----
        ident = const_pool.tile([128, 128], BF16, name="ident")
        make_identity(nc, ident)
        wq_sb = const_pool.tile([128, NE, D], BF16, name="wq_sb")
        wk_sb = const_pool.tile([128, NE, D], BF16, name="wk_sb")
        wv_sb = const_pool.tile([128, NE, D], BF16, name="wv_sb")
        nc.gpsimd.dma_start(out=wq_sb[:], in_=wq[:].rearrange("(c p) d -> p c d", p=128))
        nc.gpsimd.dma_start(out=wk_sb[:], in_=wk[:].rearrange("(c p) d -> p c d", p=128))
        nc.gpsimd.dma_start(out=wv_sb[:], in_=wv[:].rearrange("(c p) d -> p c d", p=128))
        masks_sb = const_pool.tile([128, 2, 128], BF16, name="masks_sb")
        nc.gpsimd.dma_start(out=masks_sb[:], in_=masks[:])

        # ---- x load (cast f32 -> bf16 in DMA) ----
        x_sb = []
        for lc in range(NLC):
            xt_nat = xnat_pool.tile([128, E], BF16, name=f"xnat{lc}", tag="xnat")
            nc.gpsimd.dma_start(out=xt_nat[:], in_=x[ts(lc, 128), :])
            x_sb.append(xt_nat)

        # ---- transpose x: xT[e] = [128 E, 1024 t] ----
        xT = [xt_pool.tile([128, NLC * 128], BF16, name=f"xT{e}", tag="xt")
              for e in range(NE)]
        with tc.tile_pool(name="tp_psum", bufs=2, space="PSUM") as tp_psum:
            for e in range(NE):
                for half in range(2):
                    tp = tp_psum.tile([128, 512], BF16, tag="tp")
                    for j in range(4):
                        lc = half * 4 + j
                        nc.tensor.transpose(
                            out=tp[:, ts(j, 128)],
                            in_=x_sb[lc][:, ts(e, 128)],
                            identity=ident[:],
                        )
                    nc.vector.tensor_copy(out=xT[e][:, ts(half, 512)], in_=tp[:])

            # ---- projections ----
            qT_sb = proj_pool.tile([128, NLC * 128], BF16, name="qT_sb")
            kT_loc = proj_pool.tile([128, NLC * 128], BF16, name="kT_loc")
            vT_sb = proj_pool.tile([128, NLC * 128], BF16, name="vT_sb")
            v_loc = proj_pool.tile([128, NLC * 128], BF16, name="v_loc")
            with tc.tile_pool(name="pj_psum", bufs=1, space="PSUM") as pj_psum:
                for piece in range(2):
                    k_ps = pj_psum.tile([128, 512], F32, tag="k_ps")
                    for e in range(NE):
                        nc.tensor.matmul(
                            k_ps[:], lhsT=wk_sb[:, e, :],
                            rhs=xT[e][:, ts(piece, 512)],
                            start=(e == 0), stop=(e == NE - 1),
                        )
                    nc.scalar.copy(out=kT_loc[:, ts(piece, 512)], in_=k_ps[:])
                for piece in range(2):
                    q_ps = pj_psum.tile([128, 512], F32, tag="q_ps")
                    vt_ps = pj_psum.tile([128, 512], F32, tag="vt_ps")
                    for e in range(NE):
                        nc.tensor.matmul(
                            q_ps[:], lhsT=wq_sb[:, e, :],
                            rhs=xT[e][:, ts(piece, 512)],
                            start=(e == 0), stop=(e == NE - 1),
                        )
                        nc.tensor.matmul(
                            vt_ps[:], lhsT=wv_sb[:, e, :],
                            rhs=xT[e][:, ts(piece, 512)],
                            start=(e == 0), stop=(e == NE - 1),
                        )
                    nc.vector.tensor_copy(out=qT_sb[:, ts(piece, 512)], in_=q_ps[:])
                    nc.vector.tensor_copy(out=vT_sb[:, ts(piece, 512)], in_=vt_ps[:])

                # v natural [t, D] chunks via PE transpose of vT
                for half in range(2):
                    tpv = tp_psum.tile([128, 512], BF16, tag="tp")
                    for j in range(4):
                        lc = half * 4 + j
                        nc.tensor.transpose(
                            out=tpv[:, ts(j, 128)],
                            in_=vT_sb[:, ts(lc, 128)],
                            identity=ident[:],
                        )
                    nc.vector.tensor_copy(out=v_loc[:, ts(half, 512)], in_=tpv[:])

        # ---- AllGather kT & v across the pair ----
        cc_in = dram_pool.tile([128, 2048], BF16, name="cc_in")
        cc_out = dram_pool.tile([2, 128, 2048], BF16, name="cc_out")
        nc.sync.dma_start(out=cc_in[:, ds(0, 1024)], in_=kT_loc[:])
        nc.sync.dma_start(out=cc_in[:, ds(1024, 1024)], in_=v_loc[:])
        nc.gpsimd.collective_compute(
            "AllGather", ALU.bypass, replica_groups=REPLICA_GROUPS,
            ins=[cc_in[:].opt()], outs=[cc_out[:].opt()],
        )
        kT_full = proj_pool.tile([128, 2048], BF16, name="kT_full")
        v_full = proj_pool.tile([128, 2048], BF16, name="v_full")
        for r in range(2):
            nc.sync.dma_start(out=kT_full[:, ds(r * 1024, 1024)],
                              in_=cc_out[r, :, ds(0, 1024)])
            nc.sync.dma_start(out=v_full[:, ds(r * 1024, 1024)],
                              in_=cc_out[r, :, ds(1024, 1024)])

        # ---- scores / exp / normalizer partials ----
        stats = const_pool.tile([128, NSB * 4], F32, name="stats")
        nc.vector.memset(stats[:], 0.0)
        e_tiles = {}  # (sb, lc) -> AP [128 s, 128 t]
        with tc.tile_pool(name="sc_psum", bufs=2, space="PSUM") as sc_psum:
            for sb in range(NSB):
                lo = sb // 2
                kb = kT_full[:, ds(gpos(sb) * 128, 128)]
                # masked head piece (chunk that may straddle the diagonal)
                sc = sc_psum.tile([128, 128], F32, tag="scm")
                nc.tensor.matmul(sc[:], lhsT=kb, rhs=qT_sb[:, ds(lo * 128, 128)],
                                 start=True, stop=True)
                em = e_pool.tile([128, 128], BF16, name=f"em{sb}", tag=f"em{sb}")
                nc.scalar.activation(out=em[:], in_=sc[:], func=AF.Exp, scale=SCALE)
                nc.vector.tensor_tensor(out=em[:], in0=em[:],
                                        in1=masks_sb[:, sb % 2, :], op=ALU.mult)
                nc.vector.reduce_sum(out=stats[:, ds(sb * 4 + 3, 1)], in_=em[:],
                                     axis=AX.X)
                e_tiles[(sb, lo)] = em[:]
                # full pieces
                start_lc = lo + 1
                pidx = 0
                while start_lc < NLC:
                    n = min(4, NLC - start_lc)
                    scf = sc_psum.tile([128, 512], F32, tag="scf")
                    nc.tensor.matmul(
                        scf[:, ds(0, n * 128)], lhsT=kb,
                        rhs=qT_sb[:, ds(start_lc * 128, n * 128)],
                        start=True, stop=True,
                    )
                    ef = e_pool.tile([128, n * 128], BF16,
                                     name=f"ef{sb}_{pidx}", tag=f"ef{sb}_{pidx}")
                    nc.scalar.activation(
                        out=ef[:], in_=scf[:, ds(0, n * 128)], func=AF.Exp,
                        scale=SCALE, accum_out=stats[:, ds(sb * 4 + pidx, 1)],
                    )
                    for j in range(n):
                        e_tiles[(sb, start_lc + j)] = ef[:, ts(j, 128)]
                    start_lc += n
                    pidx += 1

            # ---- normalizer AllReduce + reciprocal + v scaling ----
            zsum_loc = const_pool.tile([128, NSB], F32, name="zsum_loc")
            for sb in range(NSB):
                nc.vector.reduce_sum(out=zsum_loc[:, ds(sb, 1)],
                                     in_=stats[:, ds(sb * 4, 4)], axis=AX.X)
            zin = dram_pool.tile([128, NSB], F32, name="zin")
            zout = dram_pool.tile([128, NSB], F32, name="zout_d")
            nc.sync.dma_start(out=zin[:], in_=zsum_loc[:])
            nc.gpsimd.collective_compute(
                "AllReduce", ALU.add, replica_groups=REPLICA_GROUPS,
                ins=[zin[:].opt()], outs=[zout[:].opt()],
            )
            zsum_full = const_pool.tile([128, NSB], F32, name="zsum_full")
            nc.sync.dma_start(out=zsum_full[:], in_=zout[:])
            recip = const_pool.tile([128, NSB], F32, name="recip")
            nc.vector.reciprocal(out=recip[:], in_=zsum_full[:])
            v_scaled = proj_pool.tile([128, 2048], BF16, name="v_scaled")
            for sb in range(NSB):
                gp = gpos(sb)
                nc.vector.tensor_scalar_mul(
                    out=v_scaled[:, ds(gp * 128, 128)],
                    in0=v_full[:, ds(gp * 128, 128)],
                    scalar1=recip[:, ds(sb, 1)],
                )

            # ---- z = A @ v' per local chunk ----
            with tc.tile_pool(name="av_psum", bufs=1, space="PSUM") as av_psum:
                for lc in range(NLC):
                    zp = av_psum.tile([128, D], F32, tag="zp")
                    nsb = 2 * lc + 2
                    for sb in range(nsb):
                        nc.tensor.matmul(
                            zp[:], lhsT=e_tiles[(sb, lc)],
                            rhs=v_scaled[:, ds(gpos(sb) * 128, 128)],
                            start=(sb == 0), stop=(sb == nsb - 1),
                        )
                    z_sb = zout_pool.tile([128, D], F32, tag="z_sb")
                    if lc % 2 == 0:
                        nc.vector.tensor_copy(out=z_sb[:], in_=zp[:])
                    else:
                        nc.scalar.copy(out=z_sb[:], in_=zp[:])
                    nc.sync.dma_start(out=out[ts(lc, 128), :], in_=z_sb[:])


_NC_CACHE = None


def _get_nc():
    global _NC_CACHE
    if _NC_CACHE is None:
        _NC_CACHE = build_nc()
    return _NC_CACHE


def _host_masks(h: int) -> np.ndarray:
    tri = (np.arange(128)[None, :] >= np.arange(128)[:, None]).astype(np.float32)
    ones = np.ones((128, 128), np.float32)
    zeros = np.zeros((128, 128), np.float32)
    pair = [tri, zeros] if h == 0 else [ones, tri]
    return np.ascontiguousarray(np.stack(pair, axis=0).transpose(1, 0, 2))


def kernel(x_in, Wq, Wk, Wv):
    x_in = np.asarray(x_in, dtype=np.float32)
    Wq = np.ascontiguousarray(np.asarray(Wq, dtype=np.float32))
    Wk = np.ascontiguousarray(np.asarray(Wk, dtype=np.float32))
    Wv = np.ascontiguousarray(np.asarray(Wv, dtype=np.float32))
    nc = _get_nc()
    in_maps = []
    for c in range(N_CORES):
        b, h = c // 2, c % 2
        rows = np.concatenate(
            [x_in[b, (2 * lc + h) * 128:(2 * lc + h + 1) * 128] for lc in range(NLC)]
        )
        in_maps.append({
            "x": np.ascontiguousarray(rows),
            "wq": Wq, "wk": Wk, "wv": Wv,
            "masks": _host_masks(h),
        })
    res = run_bass_kernel_spmd(nc, in_maps, core_ids=list(range(N_CORES)))
    out = np.empty((B, T, D), np.float32)
    for c in range(N_CORES):
        b, h = c // 2, c % 2
        o = res.results[c]["out"]
        for lc in range(NLC):
            g = 2 * lc + h
            out[b, g * 128:(g + 1) * 128] = o[lc * 128:(lc + 1) * 128]
    return out



# revision 2
# speedup vs baseline: 1.3818x; 1.3818x over previous
"""Trainium2 Bass kernel for nn_AttentionHead (softmax over query axis).

Sharding: 8 cores = 4 batches x 2 halves. Core c handles batch c//2 and
row-parity h=c%2: local 128-row chunk lc <-> global chunk g=2*lc+h.

Host ships x already transposed and bf16-cast: xt[e_chunk, e, t] so the
kernel does zero PE transposes for x. Per core:
  - projections kT/vT [128 D, 1024 t] from xt chunks; vT -> v natural
    via 8 PE transposes
  - ship kT+v early: AllGather across the pair overlaps q-projection
    and local-parity score blocks
  - scores sT[s, t] = kb.T @ qT, exp (scale 1/sqrt(128)) with per-key
    column sums (softmax normalizer is over the QUERY axis), causal mask
    via host-supplied mask tiles (h=0: [tri, zeros], h=1: [ones, tri]);
    local-parity key blocks first (no collective dependency)
  - AllReduce the [128,16] normalizer partials across the pair
  - z[t, :] = sum_s E[s,t] * (v[s,:]/Z[s]); single batched output DMA
Host assembles the 8 core outputs back into [4, 2048, 128].
"""
import sys

for _p in ("/opt/trn_rl_repo",):
    if _p not in sys.path:
        sys.path.append(_p)

import numpy as np
import ml_dtypes

import concourse.bass as bass
import concourse.mybir as mybir
import concourse.tile as tile
from concourse import bacc
from concourse.bass import ds, ts
from concourse.bass_utils import run_bass_kernel_spmd
from concourse.masks import make_identity

BF16 = mybir.dt.bfloat16
F32 = mybir.dt.float32
AF = mybir.ActivationFunctionType
ALU = mybir.AluOpType
AX = mybir.AxisListType

B, T, E, D = 4, 2048, 2048, 128
NLC = 8          # local 128-row chunks per core
NE = 16          # E chunks of 128
NSB = 16         # key blocks of 128
TLOC = NLC * 128
SCALE = 1.0 / np.sqrt(D)
N_CORES = 8
REPLICA_GROUPS = [[0, 1], [2, 3], [4, 5], [6, 7]]


def build_nc():
    nc = bacc.Bacc("TRN2", target_bir_lowering=False, debug=False,
                   num_devices=N_CORES)
    xt = nc.dram_tensor("xt", [NE, 128, TLOC], BF16, kind="ExternalInput")
    wq = nc.dram_tensor("wq", [128, NE, D], BF16, kind="ExternalInput")
    wk = nc.dram_tensor("wk", [128, NE, D], BF16, kind="ExternalInput")
    wv = nc.dram_tensor("wv", [128, NE, D], BF16, kind="ExternalInput")
    masks = nc.dram_tensor("masks", [128, 2, 128], BF16, kind="ExternalInput")
    out = nc.dram_tensor("out", [TLOC, D], F32, kind="ExternalOutput")

    with tile.TileContext(nc) as tc:
        _body(nc, tc, xt, wq, wk, wv, masks, out)
    nc.compile()
    return nc


def _body(nc, tc, xt, wq, wk, wv, masks, out):
    # parity h of this core is encoded in the host-built mask tiles; the
    # kernel program itself is parity-independent.
    with (
        tc.tile_pool(name="const", bufs=1) as const_pool,
        tc.tile_pool(name="dram", bufs=1, space="DRAM") as dram_pool,
        tc.tile_pool(name="proj", bufs=1) as proj_pool,
        tc.tile_pool(name="escore", bufs=1) as e_pool,
    ):
        # ---- constants (gpsimd SWDGE path; sync ring stays free for xt) ----
        ident = const_pool.tile([128, 128], BF16, name="ident")
        make_identity(nc, ident)
        wq_sb = const_pool.tile([128, NE, D], BF16, name="wq_sb")
        wk_sb = const_pool.tile([128, NE, D], BF16, name="wk_sb")
        wv_sb = const_pool.tile([128, NE, D], BF16, name="wv_sb")
        nc.gpsimd.dma_start(out=wk_sb[:], in_=wk[:])
        nc.gpsimd.dma_start(out=wv_sb[:], in_=wv[:])
        nc.gpsimd.dma_start(out=wq_sb[:], in_=wq[:])
        masks_sb = const_pool.tile([128, 2, 128], BF16, name="masks_sb")
        nc.gpsimd.dma_start(out=masks_sb[:], in_=masks[:])

        # ---- xT load: 4 group DMAs of 4 e-chunks each ----
        xt_sb = const_pool.tile([128, NE, TLOC], BF16, name="xt_sb")
        for grp in range(4):
            nc.sync.dma_start(
                out=xt_sb[:, ds(grp * 4, 4), :],
                in_=xt[ds(grp * 4, 4), :, :].rearrange("c p t -> p c t"),
            )

        kT_loc = proj_pool.tile([128, TLOC], BF16, name="kT_loc")
        vT_sb = proj_pool.tile([128, TLOC], BF16, name="vT_sb")
        v_loc = proj_pool.tile([128, TLOC], BF16, name="v_loc")
        qT_sb = proj_pool.tile([128, TLOC], BF16, name="qT_sb")

        cc_in = dram_pool.tile([128, 2048], BF16, name="cc_in")
        cc_out = dram_pool.tile([2, 128, 2048], BF16, name="cc_out")

        with (
            tc.tile_pool(name="pj_psum", bufs=2, space="PSUM") as pj_psum,
            tc.tile_pool(name="tp_psum", bufs=2, space="PSUM") as tp_psum,
        ):
            # ---- k projection first; stage each piece for the collective ----
            for piece in range(2):
                k_ps = pj_psum.tile([128, 512], F32, tag="k_ps")
                for e in range(NE):
                    nc.tensor.matmul(
                        k_ps[:], lhsT=wk_sb[:, e, :],
                        rhs=xt_sb[:, e, ts(piece, 512)],
                        start=(e == 0), stop=(e == NE - 1),
                    )
                nc.scalar.copy(out=kT_loc[:, ts(piece, 512)], in_=k_ps[:])
                nc.sync.dma_start(out=cc_in[:, ts(piece, 512)],
                                  in_=kT_loc[:, ts(piece, 512)])
            # ---- v projection + transpose to natural [t, D] ----
            for piece in range(2):
                vt_ps = pj_psum.tile([128, 512], F32, tag="vt_ps")
                for e in range(NE):
                    nc.tensor.matmul(
                        vt_ps[:], lhsT=wv_sb[:, e, :],
                        rhs=xt_sb[:, e, ts(piece, 512)],
                        start=(e == 0), stop=(e == NE - 1),
                    )
                nc.vector.tensor_copy(out=vT_sb[:, ts(piece, 512)], in_=vt_ps[:])
            for half in range(2):
                tpv = tp_psum.tile([128, 512], BF16, tag="tp")
                for j in range(4):
                    lc = half * 4 + j
                    nc.tensor.transpose(
                        out=tpv[:, ts(j, 128)],
                        in_=vT_sb[:, ts(lc, 128)],
                        identity=ident[:],
                    )
                nc.vector.tensor_copy(out=v_loc[:, ts(half, 512)], in_=tpv[:])
                nc.sync.dma_start(out=cc_in[:, ds(1024 + half * 512, 512)],
                                  in_=v_loc[:, ts(half, 512)])

            # ---- AllGather kT & v across the pair (overlaps q + local scores) ----
            nc.gpsimd.collective_compute(
                "AllGather", ALU.bypass, replica_groups=REPLICA_GROUPS,
                ins=[cc_in[:].opt()], outs=[cc_out[:].opt()],
            )

            # ---- q projection ----
            for piece in range(2):
                q_ps = pj_psum.tile([128, 512], F32, tag="q_ps")
                for e in range(NE):
                    nc.tensor.matmul(
                        q_ps[:], lhsT=wq_sb[:, e, :],
                        rhs=xt_sb[:, e, ts(piece, 512)],
                        start=(e == 0), stop=(e == NE - 1),
                    )
                nc.scalar.copy(out=qT_sb[:, ts(piece, 512)], in_=q_ps[:])

        # remote halves of kT / v; own halves are already in SBUF.
        # cc_out rank r holds core (2b+r)'s data; this core's own rank is
        # parity h, but reading both is harmless — we read remote = 1-h via
        # host-independent trick: read BOTH ranks' halves into rank-indexed
        # tiles and select by parity at score time... parity is baked into
        # masks only. Instead read both ranks; sb chunk of parity p lives in
        # rank-p tile (g = 2*lc + r mapping).
        kT_rk = [proj_pool.tile([128, TLOC], BF16, name=f"kT_rk{r}")
                 for r in range(2)]
        v_rk = [proj_pool.tile([128, TLOC], BF16, name=f"v_rk{r}")
                for r in range(2)]
        for r in range(2):
            nc.sync.dma_start(out=kT_rk[r][:], in_=cc_out[r, :, ds(0, 1024)])
            nc.sync.dma_start(out=v_rk[r][:], in_=cc_out[r, :, ds(1024, 1024)])

        # ---- scores / exp / normalizer partials ----
        # Process key blocks in an order that puts same-parity-as-this-core
        # blocks first. Parity is host-side only, so instead order by
        # "rank": rank 0 blocks (even g) first. For core h the local blocks
        # are rank h; for h=1 cores the first 8 sbs then wait on the
        # collective. To keep the program parity-independent AND start with
        # local blocks, note kT_rk[h] is bit-identical to kT_loc, so reading
        # from kT_loc for rank-h... we cannot branch on h. Compromise:
        # alternate ranks sb = 0,1,2,... so that half the early blocks are
        # local on every core (rank sb%2, chunk sb//2).
        stats = const_pool.tile([128, NSB * 4], F32, name="stats")
        nc.vector.memset(stats[:], 0.0)
        e_tiles = {}  # (sb, lc) -> AP [128 s, 128 t]
        with tc.tile_pool(name="sc_psum", bufs=2, space="PSUM") as sc_psum:
            for sb in range(NSB):
                lo = sb // 2
                kb = kT_rk[sb % 2][:, ds(lo * 128, 128)]
                # masked head piece (chunk that may straddle the diagonal)
                sc = sc_psum.tile([128, 128], F32, tag="scm")
                nc.tensor.matmul(sc[:], lhsT=kb, rhs=qT_sb[:, ds(lo * 128, 128)],
                                 start=True, stop=True)
                em = e_pool.tile([128, 128], BF16, name=f"em{sb}", tag=f"em{sb}")
                nc.scalar.activation(out=em[:], in_=sc[:], func=AF.Exp, scale=SCALE)
                nc.vector.tensor_tensor(out=em[:], in0=em[:],
                                        in1=masks_sb[:, sb % 2, :], op=ALU.mult)
                nc.vector.reduce_sum(out=stats[:, ds(sb * 4 + 3, 1)], in_=em[:],
                                     axis=AX.X)
                e_tiles[(sb, lo)] = em[:]
                # full pieces
                start_lc = lo + 1
                pidx = 0
                while start_lc < NLC:
                    n = min(4, NLC - start_lc)
                    scf = sc_psum.tile([128, 512], F32, tag="scf")
                    nc.tensor.matmul(
                        scf[:, ds(0, n * 128)], lhsT=kb,
                        rhs=qT_sb[:, ds(start_lc * 128, n * 128)],
                        start=True, stop=True,
                    )
                    ef = e_pool.tile([128, n * 128], BF16,
                                     name=f"ef{sb}_{pidx}", tag=f"ef{sb}_{pidx}")
                    nc.scalar.activation(
                        out=ef[:], in_=scf[:, ds(0, n * 128)], func=AF.Exp,
                        scale=SCALE, accum_out=stats[:, ds(sb * 4 + pidx, 1)],
                    )
                    for j in range(n):
                        e_tiles[(sb, start_lc + j)] = ef[:, ts(j, 128)]
                    start_lc += n
                    pidx += 1

            # ---- normalizer AllReduce + reciprocal + v scaling ----
            zsum_loc = const_pool.tile([128, NSB], F32, name="zsum_loc")
            for sb in range(NSB):
                nc.vector.reduce_sum(out=zsum_loc[:, ds(sb, 1)],
                                     in_=stats[:, ds(sb * 4, 4)], axis=AX.X)
            zin = dram_pool.tile([128, NSB], F32, name="zin")
            zout = dram_pool.tile([128, NSB], F32, name="zout_d")
            nc.sync.dma_start(out=zin[:], in_=zsum_loc[:])
            nc.gpsimd.collective_compute(
                "AllReduce", ALU.add, replica_groups=REPLICA_GROUPS,
                ins=[zin[:].opt()], outs=[zout[:].opt()],
            )
            zsum_full = const_pool.tile([128, NSB], F32, name="zsum_full")
            nc.sync.dma_start(out=zsum_full[:], in_=zout[:])
            recip = const_pool.tile([128, NSB], F32, name="recip")
            nc.vector.reciprocal(out=recip[:], in_=zsum_full[:])
            v_scaled = proj_pool.tile([128, 2048], BF16, name="v_scaled")
            for sb in range(NSB):
                nc.vector.tensor_scalar_mul(
                    out=v_scaled[:, ds(sb * 128, 128)],
                    in0=v_rk[sb % 2][:, ds((sb // 2) * 128, 128)],
                    scalar1=recip[:, ds(sb, 1)],
                )

            # ---- z = A @ v' per local chunk; batched output DMA ----
            z_all = const_pool.tile([128, NLC, D], F32, name="z_all")
            with tc.tile_pool(name="av_psum", bufs=2, space="PSUM") as av_psum:
                for lc in range(NLC):
                    zp = av_psum.tile([128, D], F32, tag="zp")
                    nsb = 2 * lc + 2
                    for sb in range(nsb):
                        nc.tensor.matmul(
                            zp[:], lhsT=e_tiles[(sb, lc)],
                            rhs=v_scaled[:, ds(sb * 128, 128)],
                            start=(sb == 0), stop=(sb == nsb - 1),
                        )
                    if lc % 2 == 0:
                        nc.vector.tensor_copy(out=z_all[:, lc, :], in_=zp[:])
                    else:
                        nc.scalar.copy(out=z_all[:, lc, :], in_=zp[:])
                nc.sync.dma_start(
                    out=out[:].rearrange("(c p) d -> p c d", p=128),
                    in_=z_all[:],
                )


_NC_CACHE = None


def _get_nc():
    global _NC_CACHE
    if _NC_CACHE is None:
        _NC_CACHE = build_nc()
    return _NC_CACHE


def _host_masks(h: int) -> np.ndarray:
    tri = (np.arange(128)[None, :] >= np.arange(128)[:, None]).astype(np.float32)
    ones = np.ones((128, 128), np.float32)
    zeros = np.zeros((128, 128), np.float32)
    pair = [tri, zeros] if h == 0 else [ones, tri]
    return np.ascontiguousarray(np.stack(pair, axis=0).transpose(1, 0, 2))


def build_in_maps(x_in, Wq, Wk, Wv):
    """Host-side sharding: per-core transposed bf16 x + rearranged weights."""
    x_in = np.asarray(x_in, dtype=np.float32)
    ws = {}
    for name, W in (("wq", Wq), ("wk", Wk), ("wv", Wv)):
        W = np.asarray(W, dtype=np.float32)
        ws[name] = np.ascontiguousarray(
            W.reshape(NE, 128, D).transpose(1, 0, 2)
        ).astype(ml_dtypes.bfloat16)
    in_maps = []
    for c in range(N_CORES):
        b, h = c // 2, c % 2
        rows = np.concatenate(
            [x_in[b, (2 * lc + h) * 128:(2 * lc + h + 1) * 128]
             for lc in range(NLC)]
        )  # [1024, 2048] f32
        xt = np.ascontiguousarray(rows.T).reshape(NE, 128, TLOC)
        in_maps.append({
            "xt": xt.astype(ml_dtypes.bfloat16),
            "wq": ws["wq"], "wk": ws["wk"], "wv": ws["wv"],
            "masks": _host_masks(h).astype(ml_dtypes.bfloat16),
        })
    return in_maps


def kernel(x_in, Wq, Wk, Wv):
    nc = _get_nc()
    in_maps = build_in_maps(x_in, Wq, Wk, Wv)
    res = run_bass_kernel_spmd(nc, in_maps, core_ids=list(range(N_CORES)))
    out = np.empty((B, T, D), np.float32)
    for c in range(N_CORES):
        b, h = c // 2, c % 2
        o = res.results[c]["out"]
        for lc in range(NLC):
            g = 2 * lc + h
            out[b, g * 128:(g + 1) * 128] = o[lc * 128:(lc + 1) * 128]
    return out


# revision 5
# speedup vs baseline: 1.5080x; 1.0913x over previous
"""Trainium2 Bass kernel for nn_AttentionHead (softmax over query axis).

Sharding: 8 cores = 4 batches x 2 halves. Core c handles batch c//2 and
row-parity h=c%2: local 128-row chunk lc <-> global chunk g=2*lc+h.

Host ships x already transposed and bf16-cast: xt[e_chunk, e, t] so the
kernel does zero PE transposes for x. Per core:
  - projections kT/vT [128 D, 1024 t] from xt chunks; vT -> v natural
    via 8 PE transposes
  - ship kT+v early: AllGather across the pair overlaps q-projection
    and local-parity score blocks
  - scores sT[s, t] = kb.T @ qT, exp (scale 1/sqrt(128)) with per-key
    column sums (softmax normalizer is over the QUERY axis), causal mask
    via host-supplied mask tiles (h=0: [tri, zeros], h=1: [ones, tri]);
    local-parity key blocks first (no collective dependency)
  - AllReduce the [128,16] normalizer partials across the pair
  - z[t, :] = sum_s E[s,t] * (v[s,:]/Z[s]); single batched output DMA
Host assembles the 8 core outputs back into [4, 2048, 128].
"""
import sys

for _p in ("/opt/trn_rl_repo",):
    if _p not in sys.path:
        sys.path.append(_p)

import numpy as np
import ml_dtypes

import concourse.bass as bass
import concourse.mybir as mybir
import concourse.tile as tile
from concourse import bacc
from concourse.bass import ds, ts
from concourse.bass_utils import run_bass_kernel_spmd
from concourse.masks import make_identity

BF16 = mybir.dt.bfloat16
F32 = mybir.dt.float32
AF = mybir.ActivationFunctionType
ALU = mybir.AluOpType
AX = mybir.AxisListType

B, T, E, D = 4, 2048, 2048, 128
NLC = 8          # local 128-row chunks per core
NE = 16          # E chunks of 128
NSB = 16         # key blocks of 128
TLOC = NLC * 128
SCALE = 1.0 / np.sqrt(D)
N_CORES = 8
REPLICA_GROUPS = [[0, 1], [2, 3], [4, 5], [6, 7]]


def build_nc():
    nc = bacc.Bacc("TRN2", target_bir_lowering=False, debug=False,
                   num_devices=N_CORES)
    xt = nc.dram_tensor("xt", [NE, 128, TLOC], BF16, kind="ExternalInput")
    wq = nc.dram_tensor("wq", [128, NE, D], BF16, kind="ExternalInput")
    wk = nc.dram_tensor("wk", [128, NE, D], BF16, kind="ExternalInput")
    wv = nc.dram_tensor("wv", [128, NE, D], BF16, kind="ExternalInput")
    masks = nc.dram_tensor("masks", [128, 2, 128], BF16, kind="ExternalInput")
    out = nc.dram_tensor("out", [TLOC, D], F32, kind="ExternalOutput")

    with tile.TileContext(nc) as tc:
        _body(nc, tc, xt, wq, wk, wv, masks, out)
    nc.compile()
    return nc


def _body(nc, tc, xt, wq, wk, wv, masks, out):
    # parity h of this core is encoded in the host-built mask tiles; the
    # kernel program itself is parity-independent.
    with (
        tc.tile_pool(name="const", bufs=1) as const_pool,
        tc.tile_pool(name="dram", bufs=1, space="DRAM") as dram_pool,
        tc.tile_pool(name="proj", bufs=1) as proj_pool,
        tc.tile_pool(name="escore", bufs=1) as e_pool,
    ):
        # ---- constants (gpsimd SWDGE path; sync ring stays free for xt) ----
        ident = const_pool.tile([128, 128], BF16, name="ident")
        make_identity(nc, ident)
        wq_sb = const_pool.tile([128, NE, D], BF16, name="wq_sb")
        wk_sb = const_pool.tile([128, NE, D], BF16, name="wk_sb")
        wv_sb = const_pool.tile([128, NE, D], BF16, name="wv_sb")
        nc.gpsimd.dma_start(out=wk_sb[:], in_=wk[:])
        nc.gpsimd.dma_start(out=wv_sb[:], in_=wv[:])
        nc.gpsimd.dma_start(out=wq_sb[:], in_=wq[:])
        masks_sb = const_pool.tile([128, 2, 128], BF16, name="masks_sb")
        nc.gpsimd.dma_start(out=masks_sb[:], in_=masks[:])

        # ---- xT load: 4 group DMAs of 4 e-chunks each ----
        xt_sb = const_pool.tile([128, NE, TLOC], BF16, name="xt_sb")
        for grp in range(4):
            nc.sync.dma_start(
                out=xt_sb[:, ds(grp * 4, 4), :],
                in_=xt[ds(grp * 4, 4), :, :].rearrange("c p t -> p c t"),
            )

        kT_loc = proj_pool.tile([128, TLOC], BF16, name="kT_loc")
        vT_sb = proj_pool.tile([128, TLOC], BF16, name="vT_sb")
        v_loc = proj_pool.tile([128, TLOC], BF16, name="v_loc")
        qT_sb = proj_pool.tile([128, TLOC], BF16, name="qT_sb")

        # bar1 carries kT only; bar2 carries v + bitcast zsum partials.
        cc1_in = dram_pool.tile([128, TLOC], BF16, name="cc1_in")
        cc1_out = dram_pool.tile([2, 128, TLOC], BF16, name="cc1_out")
        cc2_in = dram_pool.tile([128, TLOC + 32], BF16, name="cc2_in")
        cc2_out = dram_pool.tile([2, 128, TLOC + 32], BF16, name="cc2_out")

        # PE warmup spin: get HAM to K=8/8 while the xt DMA streams in.
        zeros = const_pool.tile([128, 128], BF16, name="zeros")
        nc.vector.memset(zeros[:], 0.0)
        with tc.tile_pool(name="wu_psum", bufs=1, space="PSUM") as wu_psum:
            wu = wu_psum.tile([128, 128], F32, tag="wu")
            for _ in range(70):
                nc.tensor.matmul(wu[:], lhsT=zeros[:], rhs=zeros[:],
                                 start=True, stop=True)

        with (
            tc.tile_pool(name="pj_psum", bufs=2, space="PSUM") as pj_psum,
            tc.tile_pool(name="tp_psum", bufs=2, space="PSUM") as tp_psum,
        ):
            # ---- k projection first; stage each piece, then trigger bar1 ----
            for piece in range(2):
                k_ps = pj_psum.tile([128, 512], F32, tag="k_ps")
                for e in range(NE):
                    nc.tensor.matmul(
                        k_ps[:], lhsT=wk_sb[:, e, :],
                        rhs=xt_sb[:, e, ts(piece, 512)],
                        start=(e == 0), stop=(e == NE - 1),
                    )
                nc.scalar.copy(out=kT_loc[:, ts(piece, 512)], in_=k_ps[:])
                nc.sync.dma_start(out=cc1_in[:, ts(piece, 512)],
                                  in_=kT_loc[:, ts(piece, 512)])
            nc.gpsimd.collective_compute(
                "AllGather", ALU.bypass, replica_groups=REPLICA_GROUPS,
                ins=[cc1_in[:].opt()], outs=[cc1_out[:].opt()],
            )

            # ---- q, v projections + v transpose run in bar1's shadow ----
            for piece in range(2):
                q_ps = pj_psum.tile([128, 512], F32, tag="q_ps")
                for e in range(NE):
                    nc.tensor.matmul(
                        q_ps[:], lhsT=wq_sb[:, e, :],
                        rhs=xt_sb[:, e, ts(piece, 512)],
                        start=(e == 0), stop=(e == NE - 1),
                    )
                nc.scalar.copy(out=qT_sb[:, ts(piece, 512)], in_=q_ps[:])
            for piece in range(2):
                vt_ps = pj_psum.tile([128, 512], F32, tag="vt_ps")
                for e in range(NE):
                    nc.tensor.matmul(
                        vt_ps[:], lhsT=wv_sb[:, e, :],
                        rhs=xt_sb[:, e, ts(piece, 512)],
                        start=(e == 0), stop=(e == NE - 1),
                    )
                nc.vector.tensor_copy(out=vT_sb[:, ts(piece, 512)], in_=vt_ps[:])
            for half in range(2):
                tpv = tp_psum.tile([128, 512], BF16, tag="tp")
                for j in range(4):
                    lc = half * 4 + j
                    nc.tensor.transpose(
                        out=tpv[:, ts(j, 128)],
                        in_=vT_sb[:, ts(lc, 128)],
                        identity=ident[:],
                    )
                nc.vector.tensor_copy(out=v_loc[:, ts(half, 512)], in_=tpv[:])
                nc.sync.dma_start(out=cc2_in[:, ds(half * 512, 512)],
                                  in_=v_loc[:, ts(half, 512)])

        # kT halves by rank (own rank's half is bit-identical to kT_loc;
        # reading both keeps the program parity-independent).
        kT_all = proj_pool.tile([128, 2, TLOC], BF16, name="kT_all")
        nc.sync.dma_start(out=kT_all[:],
                          in_=cc1_out[:].rearrange("r p t -> p r t"))

        # ---- scores / exp / normalizer partials ----
        stats = const_pool.tile([128, NSB * 4], F32, name="stats")
        nc.vector.memset(stats[:], 0.0)
        e_tiles = {}  # (sb, lc) -> AP [128 s, 128 t]
        with tc.tile_pool(name="sc_psum", bufs=3, space="PSUM") as sc_psum:
            for sb in range(NSB):
                lo = sb // 2
                kb = kT_all[:, sb % 2, ds(lo * 128, 128)]
                # masked head piece (chunk that may straddle the diagonal)
                sc = sc_psum.tile([128, 128], F32, tag="scm")
                nc.tensor.matmul(sc[:], lhsT=kb, rhs=qT_sb[:, ds(lo * 128, 128)],
                                 start=True, stop=True)
                em = e_pool.tile([128, 128], BF16, name=f"em{sb}", tag=f"em{sb}")
                nc.scalar.activation(out=em[:], in_=sc[:], func=AF.Exp, scale=SCALE)
                nc.vector.tensor_tensor(out=em[:], in0=em[:],
                                        in1=masks_sb[:, sb % 2, :], op=ALU.mult)
                nc.vector.reduce_sum(out=stats[:, ds(sb * 4 + 3, 1)], in_=em[:],
                                     axis=AX.X)
                e_tiles[(sb, lo)] = em[:]
                # full pieces
                start_lc = lo + 1
                pidx = 0
                while start_lc < NLC:
                    n = min(4, NLC - start_lc)
                    scf = sc_psum.tile([128, 512], F32, tag="scf")
                    nc.tensor.matmul(
                        scf[:, ds(0, n * 128)], lhsT=kb,
                        rhs=qT_sb[:, ds(start_lc * 128, n * 128)],
                        start=True, stop=True,
                    )
                    ef = e_pool.tile([128, n * 128], BF16,
                                     name=f"ef{sb}_{pidx}", tag=f"ef{sb}_{pidx}")
                    nc.scalar.activation(
                        out=ef[:], in_=scf[:, ds(0, n * 128)], func=AF.Exp,
                        scale=SCALE, accum_out=stats[:, ds(sb * 4 + pidx, 1)],
                    )
                    for j in range(n):
                        e_tiles[(sb, start_lc + j)] = ef[:, ts(j, 128)]
                    start_lc += n
                    pidx += 1

            # ---- bar2: AllGather [v | bitcast zsum partials]; add locally ----
            zsum_loc = const_pool.tile([128, NSB], F32, name="zsum_loc")
            for sb in range(NSB):
                nc.vector.reduce_sum(out=zsum_loc[:, ds(sb, 1)],
                                     in_=stats[:, ds(sb * 4, 4)], axis=AX.X)
            nc.sync.dma_start(out=cc2_in[:, ds(TLOC, 32)],
                              in_=zsum_loc[:].bitcast(BF16))
            nc.gpsimd.collective_compute(
                "AllGather", ALU.bypass, replica_groups=REPLICA_GROUPS,
                ins=[cc2_in[:].opt()], outs=[cc2_out[:].opt()],
            )
            v_all = proj_pool.tile([128, 2, TLOC], BF16, name="v_all")
            nc.sync.dma_start(
                out=v_all[:],
                in_=cc2_out[:, :, ds(0, TLOC)].rearrange("r p t -> p r t"))
            zsum_pair = const_pool.tile([128, 2, NSB], F32, name="zsum_pair")
            nc.sync.dma_start(
                out=zsum_pair[:],
                in_=cc2_out[:, :, ds(TLOC, 32)].bitcast(F32)
                    .rearrange("r p t -> p r t"))
            zsum_full = const_pool.tile([128, NSB], F32, name="zsum_full")
            nc.vector.tensor_tensor(out=zsum_full[:], in0=zsum_pair[:, 0, :],
                                    in1=zsum_pair[:, 1, :], op=ALU.add)
            recip = const_pool.tile([128, NSB], F32, name="recip")
            nc.vector.reciprocal(out=recip[:], in_=zsum_full[:])
            v_scaled = proj_pool.tile([128, 2048], BF16, name="v_scaled")
            for sb in range(NSB):
                nc.vector.tensor_scalar_mul(
                    out=v_scaled[:, ds(sb * 128, 128)],
                    in0=v_all[:, sb % 2, ds((sb // 2) * 128, 128)],
                    scalar1=recip[:, ds(sb, 1)],
                )

            # ---- z = A @ v' per local chunk; batched output DMA ----
            z_all = const_pool.tile([128, NLC, D], F32, name="z_all")
            with tc.tile_pool(name="av_psum", bufs=2, space="PSUM") as av_psum:
                for lc in range(NLC):
                    zp = av_psum.tile([128, D], F32, tag="zp")
                    nsb = 2 * lc + 2
                    for sb in range(nsb):
                        nc.tensor.matmul(
                            zp[:], lhsT=e_tiles[(sb, lc)],
                            rhs=v_scaled[:, ds(sb * 128, 128)],
                            start=(sb == 0), stop=(sb == nsb - 1),
                        )
                    if lc % 2 == 0:
                        nc.vector.tensor_copy(out=z_all[:, lc, :], in_=zp[:])
                    else:
                        nc.scalar.copy(out=z_all[:, lc, :], in_=zp[:])
                nc.sync.dma_start(
                    out=out[:].rearrange("(c p) d -> p c d", p=128),
                    in_=z_all[:],
                )


_NC_CACHE = None


def _get_nc():
    global _NC_CACHE
    if _NC_CACHE is None:
        _NC_CACHE = build_nc()
    return _NC_CACHE


def _host_masks(h: int) -> np.ndarray:
    tri = (np.arange(128)[None, :] >= np.arange(128)[:, None]).astype(np.float32)
    ones = np.ones((128, 128), np.float32)
    zeros = np.zeros((128, 128), np.float32)
    pair = [tri, zeros] if h == 0 else [ones, tri]
    return np.ascontiguousarray(np.stack(pair, axis=0).transpose(1, 0, 2))


def build_in_maps(x_in, Wq, Wk, Wv):
    """Host-side sharding: per-core transposed bf16 x + rearranged weights."""
    x_in = np.asarray(x_in, dtype=np.float32)
    ws = {}
    for name, W in (("wq", Wq), ("wk", Wk), ("wv", Wv)):
        W = np.asarray(W, dtype=np.float32)
        ws[name] = np.ascontiguousarray(
            W.reshape(NE, 128, D).transpose(1, 0, 2)
        ).astype(ml_dtypes.bfloat16)
    in_maps = []
    for c in range(N_CORES):
        b, h = c // 2, c % 2
        rows = np.concatenate(
            [x_in[b, (2 * lc + h) * 128:(2 * lc + h + 1) * 128]
             for lc in range(NLC)]
        )  # [1024, 2048] f32
        xt = np.ascontiguousarray(rows.T).reshape(NE, 128, TLOC)
        in_maps.append({
            "xt": xt.astype(ml_dtypes.bfloat16),
            "wq": ws["wq"], "wk": ws["wk"], "wv": ws["wv"],
            "masks": _host_masks(h).astype(ml_dtypes.bfloat16),
        })
    return in_maps


def kernel(x_in, Wq, Wk, Wv):
    nc = _get_nc()
    in_maps = build_in_maps(x_in, Wq, Wk, Wv)
    res = run_bass_kernel_spmd(nc, in_maps, core_ids=list(range(N_CORES)))
    out = np.empty((B, T, D), np.float32)
    for c in range(N_CORES):
        b, h = c // 2, c % 2
        o = res.results[c]["out"]
        for lc in range(NLC):
            g = 2 * lc + h
            out[b, g * 128:(g + 1) * 128] = o[lc * 128:(lc + 1) * 128]
    return out


# revision 10
# speedup vs baseline: 1.5677x; 1.0396x over previous
"""Trainium2 Bass kernel for nn_AttentionHead (softmax over query axis).

Sharding: 8 cores = 4 batches x 2 halves. Core c handles batch c//2 and
row-parity h=c%2: local 128-row chunk lc <-> global chunk g=2*lc+h.

Host ships x already transposed and bf16-cast: xt[e_chunk, e, t] so the
kernel does zero PE transposes for x. Per core:
  - projections kT/vT [128 D, 1024 t] from xt chunks; vT -> v natural
    via 8 PE transposes
  - ship kT+v early: AllGather across the pair overlaps q-projection
    and local-parity score blocks
  - scores sT[s, t] = kb.T @ qT, exp (scale 1/sqrt(128)) with per-key
    column sums (softmax normalizer is over the QUERY axis), causal mask
    via host-supplied mask tiles (h=0: [tri, zeros], h=1: [ones, tri]);
    local-parity key blocks first (no collective dependency)
  - AllReduce the [128,16] normalizer partials across the pair
  - z[t, :] = sum_s E[s,t] * (v[s,:]/Z[s]); single batched output DMA
Host assembles the 8 core outputs back into [4, 2048, 128].
"""
import sys

for _p in ("/opt/trn_rl_repo",):
    if _p not in sys.path:
        sys.path.append(_p)

import numpy as np
import ml_dtypes

import concourse.bass as bass
import concourse.mybir as mybir
import concourse.tile as tile
from concourse import bacc
from concourse.bass import ds, ts
from concourse.bass_utils import run_bass_kernel_spmd
from concourse.masks import make_identity

BF16 = mybir.dt.bfloat16
F32 = mybir.dt.float32
AF = mybir.ActivationFunctionType
ALU = mybir.AluOpType
AX = mybir.AxisListType

B, T, E, D = 4, 2048, 2048, 128
NLC = 8          # local 128-row chunks per core
NE = 16          # E chunks of 128
NSB = 16         # key blocks of 128
TLOC = NLC * 128
SCALE = 1.0 / np.sqrt(D)
N_CORES = 8
REPLICA_GROUPS = [[0, 1], [2, 3], [4, 5], [6, 7]]


def build_nc():
    nc = bacc.Bacc("TRN2", target_bir_lowering=False, debug=False,
                   num_devices=N_CORES)
    xt = nc.dram_tensor("xt", [NE, 128, TLOC], BF16, kind="ExternalInput")
    wq = nc.dram_tensor("wq", [128, NE, D], BF16, kind="ExternalInput")
    wk = nc.dram_tensor("wk", [128, NE, D], BF16, kind="ExternalInput")
    wv = nc.dram_tensor("wv", [128, NE, D], BF16, kind="ExternalInput")
    masks = nc.dram_tensor("masks", [128, 2, 128], BF16, kind="ExternalInput")
    out = nc.dram_tensor("out", [TLOC, D], F32, kind="ExternalOutput")

    with tile.TileContext(nc) as tc:
        _body(nc, tc, xt, wq, wk, wv, masks, out)
    nc.compile()
    return nc


def _body(nc, tc, xt, wq, wk, wv, masks, out):
    # parity h of this core is encoded in the host-built mask tiles; the
    # kernel program itself is parity-independent.
    with (
        tc.tile_pool(name="const", bufs=1) as const_pool,
        tc.tile_pool(name="dram", bufs=1, space="DRAM") as dram_pool,
        tc.tile_pool(name="proj", bufs=1) as proj_pool,
        tc.tile_pool(name="escore", bufs=1) as e_pool,
    ):
        # ---- constants (gpsimd SWDGE path; sync ring stays free for xt) ----
        ident = const_pool.tile([128, 128], BF16, name="ident")
        make_identity(nc, ident)
        wq_sb = const_pool.tile([128, NE, D], BF16, name="wq_sb")
        wk_sb = const_pool.tile([128, NE, D], BF16, name="wk_sb")
        wv_sb = const_pool.tile([128, NE, D], BF16, name="wv_sb")
        nc.gpsimd.dma_start(out=wk_sb[:], in_=wk[:])
        nc.gpsimd.dma_start(out=wv_sb[:], in_=wv[:])
        nc.gpsimd.dma_start(out=wq_sb[:], in_=wq[:])
        masks_sb = const_pool.tile([128, 2, 128], BF16, name="masks_sb")
        nc.gpsimd.dma_start(out=masks_sb[:], in_=masks[:])

        # ---- xT load: 4 group DMAs of 4 e-chunks each ----
        xt_sb = const_pool.tile([128, NE, TLOC], BF16, name="xt_sb")
        for grp in range(4):
            nc.sync.dma_start(
                out=xt_sb[:, ds(grp * 4, 4), :],
                in_=xt[ds(grp * 4, 4), :, :].rearrange("c p t -> p c t"),
            )

        kT_loc = proj_pool.tile([128, TLOC], BF16, name="kT_loc")
        vT_sb = proj_pool.tile([128, TLOC], BF16, name="vT_sb")
        v_loc = proj_pool.tile([128, TLOC], BF16, name="v_loc")
        qT_sb = proj_pool.tile([128, TLOC], BF16, name="qT_sb")

        # bar1 carries kT; bar1.5 carries v; bar2 is the zsum AllReduce.
        cc1_in = dram_pool.tile([128, TLOC], BF16, name="cc1_in")
        cc1_out = dram_pool.tile([2, 128, TLOC], BF16, name="cc1_out")
        cc2_in = dram_pool.tile([128, TLOC], BF16, name="cc2_in")
        cc2_out = dram_pool.tile([2, 128, TLOC], BF16, name="cc2_out")
        zin = dram_pool.tile([128, NSB], F32, name="zin")
        zout = dram_pool.tile([128, NSB], F32, name="zout_d")

        # PE warmup spin: get HAM to K=8/8 while the xt DMA streams in.
        zeros = const_pool.tile([128, 128], BF16, name="zeros")
        nc.vector.memset(zeros[:], 0.0)
        with tc.tile_pool(name="wu_psum", bufs=1, space="PSUM") as wu_psum:
            wu = wu_psum.tile([128, 128], F32, tag="wu")
            for _ in range(70):
                nc.tensor.matmul(wu[:], lhsT=zeros[:], rhs=zeros[:],
                                 start=True, stop=True)

        with (
            tc.tile_pool(name="pj_psum", bufs=2, space="PSUM") as pj_psum,
            tc.tile_pool(name="tp_psum", bufs=2, space="PSUM") as tp_psum,
        ):
            # ---- k projection first; stage each piece, then trigger bar1 ----
            for piece in range(2):
                k_ps = pj_psum.tile([128, 512], F32, tag="k_ps")
                for e in range(NE):
                    nc.tensor.matmul(
                        k_ps[:], lhsT=wk_sb[:, e, :],
                        rhs=xt_sb[:, e, ts(piece, 512)],
                        start=(e == 0), stop=(e == NE - 1),
                    )
                nc.scalar.copy(out=kT_loc[:, ts(piece, 512)], in_=k_ps[:])
                nc.sync.dma_start(out=cc1_in[:, ts(piece, 512)],
                                  in_=kT_loc[:, ts(piece, 512)])
            nc.gpsimd.collective_compute(
                "AllGather", ALU.bypass, replica_groups=REPLICA_GROUPS,
                ins=[cc1_in[:].opt()], outs=[cc1_out[:].opt()],
            )

            # ---- q, v projections + v transpose run in bar1's shadow ----
            for piece in range(2):
                q_ps = pj_psum.tile([128, 512], F32, tag="q_ps")
                for e in range(NE):
                    nc.tensor.matmul(
                        q_ps[:], lhsT=wq_sb[:, e, :],
                        rhs=xt_sb[:, e, ts(piece, 512)],
                        start=(e == 0), stop=(e == NE - 1),
                    )
                nc.scalar.copy(out=qT_sb[:, ts(piece, 512)], in_=q_ps[:])
            for piece in range(2):
                vt_ps = pj_psum.tile([128, 512], F32, tag="vt_ps")
                for e in range(NE):
                    nc.tensor.matmul(
                        vt_ps[:], lhsT=wv_sb[:, e, :],
                        rhs=xt_sb[:, e, ts(piece, 512)],
                        start=(e == 0), stop=(e == NE - 1),
                    )
                nc.vector.tensor_copy(out=vT_sb[:, ts(piece, 512)], in_=vt_ps[:])
            for half in range(2):
                tpv = tp_psum.tile([128, 512], BF16, tag="tp")
                for j in range(4):
                    lc = half * 4 + j
                    nc.tensor.transpose(
                        out=tpv[:, ts(j, 128)],
                        in_=vT_sb[:, ts(lc, 128)],
                        identity=ident[:],
                    )
                nc.vector.tensor_copy(out=v_loc[:, ts(half, 512)], in_=tpv[:])
                nc.sync.dma_start(out=cc2_in[:, ds(half * 512, 512)],
                                  in_=v_loc[:, ts(half, 512)])
            nc.gpsimd.collective_compute(
                "AllGather", ALU.bypass, replica_groups=REPLICA_GROUPS,
                ins=[cc2_in[:].opt()], outs=[cc2_out[:].opt()],
            )

        # kT halves by rank (own rank's half is bit-identical to kT_loc;
        # reading both keeps the program parity-independent).
        kT_all = proj_pool.tile([128, 2, TLOC], BF16, name="kT_all")
        nc.sync.dma_start(out=kT_all[:],
                          in_=cc1_out[:].rearrange("r p t -> p r t"))
        v_all = proj_pool.tile([128, 2, TLOC], BF16, name="v_all")
        nc.sync.dma_start(out=v_all[:],
                          in_=cc2_out[:].rearrange("r p t -> p r t"))

        # ---- scores / exp / normalizer partials ----
        stats = const_pool.tile([128, NSB * 4], F32, name="stats")
        zsum_loc = const_pool.tile([128, NSB], F32, name="zsum_loc")
        nc.vector.memset(stats[:], 0.0)
        e_tiles = {}  # (sb, lc) -> AP [128 s, 128 t]
        with tc.tile_pool(name="sc_psum", bufs=3, space="PSUM") as sc_psum:
            # diagonal-straddling blocks, batched 4 per exp
            for grp in range(4):
                dg = sc_psum.tile([128, 512], F32, tag="dgm")
                for j in range(4):
                    sb = grp * 4 + j
                    lo = sb // 2
                    nc.tensor.matmul(
                        dg[:, ts(j, 128)], lhsT=kT_all[:, sb % 2, ds(lo * 128, 128)],
                        rhs=qT_sb[:, ds(lo * 128, 128)], start=True, stop=True)
                em4 = e_pool.tile([128, 512], BF16, name=f"em4_{grp}",
                                  tag=f"em4_{grp}")
                nc.scalar.activation(out=em4[:], in_=dg[:], func=AF.Exp,
                                     scale=SCALE)
                for half in range(2):
                    nc.vector.tensor_tensor(
                        out=em4[:, ts(half, 256)], in0=em4[:, ts(half, 256)],
                        in1=masks_sb[:].rearrange("p a b -> p (a b)"),
                        op=ALU.mult)
                for j in range(4):
                    sb = grp * 4 + j
                    nc.vector.reduce_sum(out=stats[:, ds(sb * 4 + 3, 1)],
                                         in_=em4[:, ts(j, 128)], axis=AX.X)
                    e_tiles[(sb, sb // 2)] = em4[:, ts(j, 128)]
            # full blocks per key chunk, with accumulated column sums
            for sb in range(NSB):
                lo = sb // 2
                kb = kT_all[:, sb % 2, ds(lo * 128, 128)]
                start_lc = lo + 1
                pidx = 0
                while start_lc < NLC:
                    n = min(4, NLC - start_lc)
                    scf = sc_psum.tile([128, 512], F32, tag="scf")
                    nc.tensor.matmul(
                        scf[:, ds(0, n * 128)], lhsT=kb,
                        rhs=qT_sb[:, ds(start_lc * 128, n * 128)],
                        start=True, stop=True,
                    )
                    ef = e_pool.tile([128, n * 128], BF16,
                                     name=f"ef{sb}_{pidx}", tag=f"ef{sb}_{pidx}")
                    nc.scalar.activation(
                        out=ef[:], in_=scf[:, ds(0, n * 128)], func=AF.Exp,
                        scale=SCALE, accum_out=stats[:, ds(sb * 4 + pidx, 1)],
                    )
                    for j in range(n):
                        e_tiles[(sb, start_lc + j)] = ef[:, ts(j, 128)]
                    start_lc += n
                    pidx += 1
                nc.vector.reduce_sum(out=zsum_loc[:, ds(sb, 1)],
                                     in_=stats[:, ds(sb * 4, 4)], axis=AX.X)

            # ---- bar2: zsum AllReduce + reciprocal + v scaling ----
            nc.sync.dma_start(out=zin[:], in_=zsum_loc[:])
            nc.gpsimd.collective_compute(
                "AllReduce", ALU.add, replica_groups=REPLICA_GROUPS,
                ins=[zin[:].opt()], outs=[zout[:].opt()],
            )
            zsum_full = const_pool.tile([128, NSB], F32, name="zsum_full")
            nc.sync.dma_start(out=zsum_full[:], in_=zout[:])
            recip = const_pool.tile([128, NSB], F32, name="recip")
            nc.vector.reciprocal(out=recip[:], in_=zsum_full[:])
            v_scaled = proj_pool.tile([128, 2048], BF16, name="v_scaled")
            for sb in range(NSB):
                nc.vector.tensor_scalar_mul(
                    out=v_scaled[:, ds(sb * 128, 128)],
                    in0=v_all[:, sb % 2, ds((sb // 2) * 128, 128)],
                    scalar1=recip[:, ds(sb, 1)],
                )

            # ---- z = A @ v' per local chunk; batched output DMA ----
            z_all = const_pool.tile([128, NLC, D], F32, name="z_all")
            with tc.tile_pool(name="av_psum", bufs=2, space="PSUM") as av_psum:
                for lc in range(NLC):
                    zp = av_psum.tile([128, D], F32, tag="zp")
                    nsb = 2 * lc + 2
                    for sb in range(nsb):
                        nc.tensor.matmul(
                            zp[:], lhsT=e_tiles[(sb, lc)],
                            rhs=v_scaled[:, ds(sb * 128, 128)],
                            start=(sb == 0), stop=(sb == nsb - 1),
                        )
                    if lc % 2 == 0:
                        nc.vector.tensor_copy(out=z_all[:, lc, :], in_=zp[:])
                    else:
                        nc.scalar.copy(out=z_all[:, lc, :], in_=zp[:])
                    if lc == 3 or lc == 7:
                        nc.sync.dma_start(
                            out=out[ds((lc // 4) * 512, 512), :]
                                .rearrange("(c p) d -> p c d", p=128),
                            in_=z_all[:, ds((lc // 4) * 4, 4), :],
                        )


_NC_CACHE = None


def _get_nc():
    global _NC_CACHE
    if _NC_CACHE is None:
        _NC_CACHE = build_nc()
    return _NC_CACHE


def _host_masks(h: int) -> np.ndarray:
    tri = (np.arange(128)[None, :] >= np.arange(128)[:, None]).astype(np.float32)
    ones = np.ones((128, 128), np.float32)
    zeros = np.zeros((128, 128), np.float32)
    pair = [tri, zeros] if h == 0 else [ones, tri]
    return np.ascontiguousarray(np.stack(pair, axis=0).transpose(1, 0, 2))


def build_in_maps(x_in, Wq, Wk, Wv):
    """Host-side sharding: per-core transposed bf16 x + rearranged weights."""
    x_in = np.asarray(x_in, dtype=np.float32)
    ws = {}
    for name, W in (("wq", Wq), ("wk", Wk), ("wv", Wv)):
        W = np.asarray(W, dtype=np.float32)
        ws[name] = np.ascontiguousarray(
            W.reshape(NE, 128, D).transpose(1, 0, 2)
        ).astype(ml_dtypes.bfloat16)
    in_maps = []
    for c in range(N_CORES):
        b, h = c // 2, c % 2
        rows = np.concatenate(
            [x_in[b, (2 * lc + h) * 128:(2 * lc + h + 1) * 128]
             for lc in range(NLC)]
        )  # [1024, 2048] f32
        xt = np.ascontiguousarray(rows.T).reshape(NE, 128, TLOC)
        in_maps.append({
            "xt": xt.astype(ml_dtypes.bfloat16),
            "wq": ws["wq"], "wk": ws["wk"], "wv": ws["wv"],
            "masks": _host_masks(h).astype(ml_dtypes.bfloat16),
        })
    return in_maps


def kernel(x_in, Wq, Wk, Wv):
    nc = _get_nc()
    in_maps = build_in_maps(x_in, Wq, Wk, Wv)
    res = run_bass_kernel_spmd(nc, in_maps, core_ids=list(range(N_CORES)))
    out = np.empty((B, T, D), np.float32)
    for c in range(N_CORES):
        b, h = c // 2, c % 2
        o = res.results[c]["out"]
        for lc in range(NLC):
            g = 2 * lc + h
            out[b, g * 128:(g + 1) * 128] = o[lc * 128:(lc + 1) * 128]
    return out
